# revision 21
# baseline (speedup 1.0000x reference)
"""DenseGATv2 layer on 8 Trainium2 NeuronCores (Bass/Tile).

Math: the reference computes, per head,
    e[i,j]  = leaky_relu(s_i[i] + s_j[j], 0.2)   (s_i = h@a_src, s_j = h@a_dst)
    attn    = softmax_j(where(adj[i,j], e, -9e15))
    out[i]  = attn @ h
Since exp is monotonic and softmax is scale-invariant per row i:
    exp(leaky_relu(s_i+s_j)) * exp(-0.2 s_i) = max(exp(s_j + 0.8 s_i), exp(0.2 s_j))
and the row-constant exp(-0.2 s_i) cancels in the softmax normalization.  With
per-node precomputes rep_i = exp(0.8 s_i) (replicated across partitions),
rv_j = exp(s_j) and v_j = exp(0.2 s_j) (per-partition scalars), the whole
masked softmax numerator for one (j-chunk, head) tile is:
    P'[j,i] = max(rep_i * rv_j, v_j)        one tensor_scalar   (bf16, 4x)
    Pm      = P' * mask[j,i]                one tensor_tensor   (bf16, 2x),
                                            4 heads stacked against a
                                            stride-0-repeat mask AP
— no dense exp/leaky passes on ScalarE at all.  An appended ones-column in the
aggregation operand yields the softmax denominator inside the same PE matmuls
that aggregate h (attention tile stationary, so the output lands
destination-rows-on-partitions and phase 2 is just reciprocal + scale).

Sharding: destination rows i split across 8 cores (512 rows each); every core
computes the full h = x @ [W | W@a_src | W@a_dst] locally (one 128-deep matmul
per j-chunk) and reduces over all 4096 source nodes j for its own rows.

Trn2 scheduling notes: walrus allows at most ONE hardware sync-wait per
engine instruction (extras must be legalized into EventSemaphore ops by
Bacc.finalize, which this kernel relies on).  To keep that legalization
cheap the kernel also ships all bulk inputs as a single concatenated
tensor (one DMA -> one queue semaphore) and drains h PSUM with one engine.
PSUM output accumulators are pre-zeroed with memset and accumulated with
start=False throughout: interleaved per-head accumulation regions sharing
a PSUM bank corrupt each other's first contribution when start=True zeroing
is used per region (observed on HW: last-written head exact, others short).
"""

import numpy as np
import ml_dtypes

import concourse.bass as bass
import concourse.tile as tile
from concourse.bacc import Bacc
from concourse import mybir
from concourse.bass_utils import run_bass_kernel_spmd

bf16 = ml_dtypes.bfloat16

N, IN_DIM, HEADS, OUT_DIM = 4096, 128, 4, 64
NCORES, ROWS = 8, N // 8          # 512 dest rows per core
P = 128                           # partitions
C = N // P                        # 32 j-chunks
OWNC = ROWS // P                  # 4 own i-chunks per core
COLS = 2 * IN_DIM + 2 * HEADS     # 264 = 256 h cols + 4 s_src + 4 s_dst
DAUG = OUT_DIM + 1                # 65: head h-slice + ones column
BULK = ROWS + COLS + N            # xownT | W_aug | xT columns

_cache = {}


def _build_bass(repeat=1):
    nc = Bacc()
    f32 = mybir.dt.float32
    f16 = mybir.dt.float16
    bfl = mybir.dt.bfloat16
    Act = mybir.ActivationFunctionType
    Alu = mybir.AluOpType

    bulk = nc.declare_dram_parameter("bulk", [P, BULK], f32, isOutput=False)
    maskT = nc.declare_dram_parameter("maskT", [N, ROWS], bfl, isOutput=False)
    out = nc.declare_dram_parameter("out", [ROWS, HEADS * OUT_DIM], f32, isOutput=True)
    riT_dram = nc.dram_tensor("riT_scratch", [OWNC * HEADS, P], bfl)

    with tile.TileContext(nc) as tc:
        with (
            tc.tile_pool(name="consts", bufs=1) as consts,
            tc.tile_pool(name="hb", bufs=C) as hb_pool,
            tc.tile_pool(name="vr", bufs=C) as vr_pool,
            tc.tile_pool(name="mask", bufs=6) as mask_pool,
            tc.tile_pool(name="tt", bufs=3) as t_pool,
            tc.tile_pool(name="pm", bufs=3) as pm_pool,
            tc.tile_pool(name="fin", bufs=4) as fin_pool,
            tc.tile_pool(name="psout", bufs=1, space="PSUM") as ps_out_pool,
            tc.tile_pool(name="ps_h", bufs=3, space="PSUM") as ps_h_pool,
            tc.tile_pool(name="ps_s", bufs=1, space="PSUM") as ps_s_pool,
        ):
          for _rep in range(repeat):
            # per-own-chunk output accumulators: claim PSUM banks first so they
            # are never aliased with the h-matmul banks (no cross-pool WAW).
            ps_out = [ps_out_pool.tile([P, HEADS, DAUG], f32, tag=f"po{k}", name=f"ps_out{k}")
                      for k in range(OWNC)]
            for k in range(OWNC):
                nc.vector.memset(ps_out[k][:, :, :], 0.0)

            # ---- all bulk inputs in ONE DMA -> one queue semaphore
            sb_bulk = consts.tile([P, BULK], f32, tag="sb_bulk")
            nc.sync.dma_start(out=sb_bulk, in_=bulk[:, :])
            sb_xown = sb_bulk[:, 0:ROWS]
            sb_W = sb_bulk[:, ROWS:ROWS + COLS]
            sb_xT = sb_bulk[:, ROWS + COLS:BULK]
            w_sd = sb_bulk[:, ROWS + 2 * IN_DIM:ROWS + 2 * IN_DIM + HEADS]

            # ---- phase 0b: r_i = exp(0.8 s_src) for own rows, replicated
            # across partitions via DMA transpose + DRAM-bounce broadcast.
            ps_sown = ps_s_pool.tile([P, COLS], f32, tag="ps_s", name="ps_sown")
            for oc in range(OWNC):
                nc.tensor.matmul(
                    ps_sown[:, oc * HEADS:(oc + 1) * HEADS],
                    sb_xown[:, oc * P:(oc + 1) * P], w_sd,
                    start=True, stop=True,
                )
            vown = consts.tile([P, P], bfl, tag="vown")
            nc.vector.memset(vown, 0.0)
            nc.scalar.activation(vown[:, 0:OWNC * HEADS], ps_sown[:, 0:OWNC * HEADS],
                                 Act.Exp, scale=0.8)
            vT = consts.tile([P, P], bfl, tag="vT")
            nc.sync.dma_start(out=vT, in_=vown, transpose=True)
            nc.sync.dma_start(out=riT_dram[:, :], in_=vT[0:OWNC * HEADS, :])
            sb_rep = consts.tile([P, HEADS, ROWS], bfl, tag="sb_rep")
            for hd in range(HEADS):
                for oc in range(OWNC):
                    row = riT_dram[oc * HEADS + hd:oc * HEADS + hd + 1, :]
                    bcast = bass.AP(tensor=row.tensor, offset=row.offset,
                                    ap=[[0, P], row.ap[-1]])
                    nc.sync.dma_start(out=sb_rep[:, hd, oc * P:(oc + 1) * P], in_=bcast)

            # ---- phase 0c: h_aug per j-chunk; PSUM drained by VectorE only
            hb = []
            vr = []
            for c in range(C):
                ps_h = ps_h_pool.tile([P, COLS], f32, tag="ps_h")
                nc.tensor.matmul(ps_h, sb_xT[:, c * P:(c + 1) * P], sb_W,
                                 start=True, stop=True)
                hb_c = hb_pool.tile([P, HEADS, DAUG], bfl, tag="hb")
                nc.vector.memset(hb_c[:, :, OUT_DIM:DAUG], 1.0)
                nc.scalar.activation(
                    hb_c[:, :, 0:OUT_DIM],
                    ps_h[:, 0:2 * IN_DIM].rearrange("p (h d) -> p h d", h=HEADS),
                    Act.Copy,
                )
                s16_c = vr_pool.tile([P, 2 * HEADS], f16, tag="s16")
                nc.scalar.activation(s16_c, ps_h[:, 2 * IN_DIM:COLS], Act.Copy)
                vr_c = vr_pool.tile([P, 2, HEADS], f32, tag="vr")
                nc.scalar.activation(vr_c[:, 0, :], s16_c[:, HEADS:2 * HEADS], Act.Exp, scale=0.2)
                nc.scalar.activation(vr_c[:, 1, :], s16_c[:, HEADS:2 * HEADS], Act.Exp, scale=1.0)
                hb.append(hb_c)
                vr.append(vr_c)

            # ---- phase 1: hot loop over j-chunks
            for c in range(C):
                mask_c = mask_pool.tile([P, ROWS], bfl, tag="mask")
                nc.sync.dma_start(out=mask_c, in_=maskT[c * P:(c + 1) * P, :])
                t_all = t_pool.tile([P, HEADS, ROWS], bfl, tag="T")
                for hd in range(HEADS):
                    nc.vector.tensor_scalar(
                        out=t_all[:, hd, :], in0=sb_rep[:, hd, :],
                        scalar1=vr[c][:, 1, hd:hd + 1],
                        scalar2=vr[c][:, 0, hd:hd + 1],
                        op0=Alu.mult, op1=Alu.max,
                    )
                pm_all = pm_pool.tile([P, HEADS, ROWS], bfl, tag="pm")
                mask_rep = bass.AP(tensor=mask_c.tensor, offset=mask_c.offset,
                                   ap=[mask_c.ap[0], [0, HEADS], mask_c.ap[-1]])
                nc.vector.tensor_tensor(out=pm_all, in0=t_all, in1=mask_rep,
                                        op=Alu.mult)
                for hd in range(HEADS):
                    for k in range(OWNC):
                        nc.tensor.matmul(
                            ps_out[k][:, hd, :],
                            pm_all[:, hd, k * P:(k + 1) * P], hb[c][:, hd, :],
                            start=False, stop=(c == C - 1),
                            skip_group_check=True,
                        )

            # ---- phase 2: normalize + store (dest rows already on partitions)
            for k in range(OWNC):
                out_k = fin_pool.tile([P, HEADS, OUT_DIM], f32, tag="outk")
                for hd in range(HEADS):
                    rcp = fin_pool.tile([P, 1], f32, tag="rcp")
                    nc.vector.reciprocal(rcp, ps_out[k][:, hd, OUT_DIM:DAUG])
                    nc.vector.tensor_scalar(
                        out=out_k[:, hd, :], in0=ps_out[k][:, hd, 0:OUT_DIM],
                        scalar1=rcp, scalar2=None, op0=Alu.mult,
                    )
                nc.sync.dma_start(
                    out=out[k * P:(k + 1) * P, :].rearrange("p (h d) -> p h d", h=HEADS),
                    in_=out_k,
                )
    nc.finalize()
    return nc


def _prep_in_maps(x, adj_mask, W_lin, a_src, a_dst):

    W_lin = np.asarray(W_lin, np.float32)
    W3 = W_lin.reshape(IN_DIM, HEADS, OUT_DIM).astype(np.float64)
    W_src = (W3 @ np.asarray(a_src, np.float64)).astype(np.float32)
    W_dst = (W3 @ np.asarray(a_dst, np.float64)).astype(np.float32)
    W_aug = np.concatenate([W_lin, W_src, W_dst], axis=1)
    x = np.asarray(x, np.float32)
    xT = np.ascontiguousarray(x.T)
    adj = np.asarray(adj_mask, bool)
    maskT = np.where(adj.T, np.float32(1.0), np.float32(0.0)).astype(bf16)

    in_maps = []
    for core in range(NCORES):
        sl = slice(core * ROWS, (core + 1) * ROWS)
        bulk = np.ascontiguousarray(
            np.concatenate([xT[:, sl], W_aug, xT], axis=1))
        in_maps.append({
            "bulk": bulk,
            "maskT": np.ascontiguousarray(maskT[:, sl]),
        })

    return in_maps


def kernel(x, adj_mask, W_lin, a_src, a_dst):
    if "nc" not in _cache:
        _cache["nc"] = _build_bass()
    nc = _cache["nc"]
    in_maps = _prep_in_maps(x, adj_mask, W_lin, a_src, a_dst)
    res = run_bass_kernel_spmd(nc, in_maps, core_ids=list(range(NCORES)))
    outs = [r["out"] for r in res.results]
    return np.concatenate(outs, axis=0).astype(np.float32)
